# revision 13
# baseline (speedup 1.0000x reference)
"""Trainium2 kernel for AdjaEdgeNorm: per-destination-node edge-mailbox
normalization (mean/std over each dst node's incoming edge features).

Strategy (follows the sharding hint):
  - Host: partition the graph by destination node. Nodes are sorted by
    degree (desc) and dealt round-robin to the 8 cores, so every core has
    the same degree profile. Each core's 6250 nodes are grouped into 49
    regions of 128 nodes; a region's nodes are padded to the region max
    degree C_r rounded up to a multiple of 2. Each core's data is one
    [128, sum_r C_r*64] int8 matrix (q = round(32*x); the scale cancels
    in the normalization): partition p of region r holds node (r,p)'s
    padded edge mailbox, flattened.
  - Device (SPMD, one NEFF on 8 cores, zero cross-core communication),
    two chunk tiers balancing ACT vs DVE vs DMA:
      B-chunks (pure-ACT): raw int8 HWDGE load; ACT Square+accum (sum of
        squares, int8 read directly) and ACT Identity with per-node AP
        scale/bias -> int8 normalized output (fused quantize, RNE); DVE
        does only the row-sum via one fused scalar_tensor_tensor
        (left half + right half, accumulated).
      A-chunks (pure-DVE): SWDGE int8->bf16 cast load; DVE STT row-sum,
        STT self-mult sum-of-squares, tensor_scalar (sub,mult) normalize
        out-of-place -> bf16 (4x mode); SWDGE cast bf16->int8 store.
    Per-chunk stats on DVE with host-precomputed count-correction
    constants; sqrt(var + delta) on ACT replaces max(var,0)+eps (the
    reference's +eps only matters for degenerate nodes whose output is 0
    either way).
  - Host: gather int8 output back to edge order; apply gamma/beta (and
    the 1/32 dequant) as an exact elementwise epilogue.
"""

import sys
import types

import numpy as np

N_NODES = 50000
N_EDGES = 1600000
F = 64
EPS = 1e-5
QSCALE = 32.0
OSCALE = 32.0
NCORES = 8
P = 128
NODES_PER_CORE = N_NODES // NCORES          # 6250
NREG = (NODES_PER_CORE + P - 1) // P        # 49
CHUNK_W_MAX = 8192                          # elems/partition per chunk DMA
IO_BUFS_A = 5
IO_BUFS_B = 5
OUT_BUFS_A = 3
OUT_BUFS_B = 3
ALPHA = 0.45                                # fraction of data on the A (DVE) tier
VAR_DELTA = 1.0                             # sqrt(var + delta) guard, q^2 units

_PLAN_CACHE = {}
_BUILD_CACHE = {}


def _bf16():
    import ml_dtypes
    return np.dtype(ml_dtypes.bfloat16)


def _install_ntff_hook():
    """The agent container's antenv stub lacks axon_hooks; recreate it so
    run_bass_kernel_spmd(trace=True) can capture NTFF profiles. Harmless
    if unavailable."""
    if "antenv.axon_hooks" in sys.modules:
        return
    try:
        from trn_agent_boot.trn_boot import _ntff_profile_via_ctypes
        hook = _ntff_profile_via_ctypes("/opt/axon/libaxon_pjrt.so")
    except Exception:
        hook = None
    mod = types.ModuleType("antenv.axon_hooks")
    mod.get_axon_ntff_profile_hook = lambda: hook
    mod.set_axon_ntff_profile_hook = lambda h: None
    sys.modules["antenv.axon_hooks"] = mod


def _split_multiwaits(nc):
    """walrus in this container supports a single sync-wait per instruction;
    Tile's tail drain can carry one wait per DMA lane. Hoist extras onto
    standalone NoOps on the same engine, just before the instruction."""
    import concourse.mybir as mybir

    k = 0
    for f in nc.m.functions:
        for bb in f.blocks:
            new = []
            for inst in bb.instructions:
                si = inst.sync_info
                if si is not None and si.on_wait is not None and len(si.on_wait) > 1:
                    for w in si.on_wait[:-1]:
                        nop = mybir.InstNoOp(name=f"I-mwsplit-{k}", ins=[], outs=[])
                        k += 1
                        nop.engine = inst.engine
                        nop.sync_info = mybir.SyncInfo(on_wait=[w], on_update=[])
                        new.append(nop)
                    si.on_wait = si.on_wait[-1:]
                new.append(inst)
            bb.instructions[:] = new


def _plan(dst):
    """All index preprocessing derived from dst alone."""
    dst = np.asarray(dst, dtype=np.int64)
    deg = np.bincount(dst, minlength=N_NODES)
    order = np.argsort(-deg, kind="stable")          # node ids, degree desc
    dsort = deg[order]

    # Region widths: region r spans global degree-ranks [1024r, 1024r+1024).
    # Rounded up to a multiple of 2 so the fused half+half row-sum stays
    # region-local.
    C = np.empty(NREG, np.int64)
    for r in range(NREG):
        c = max(int(dsort[min(1024 * r, N_NODES - 1)]), 1)
        C[r] = (c + 1) // 2 * 2
    regoff64 = np.zeros(NREG + 1, np.int64)
    np.cumsum(C, out=regoff64[1:])                   # region start, 64-blocks
    F64 = int(regoff64[-1])
    F_total = F64 * F

    # Chunks: consecutive regions grouped so each chunk DMA is big.
    chunks = []  # (list_of_regions, off_floats, width_floats)
    cur, w = [], 0
    for r in range(NREG):
        wr = int(C[r]) * F
        if w + wr > CHUNK_W_MAX and cur:
            chunks.append((cur, int(regoff64[cur[0]]) * F, w))
            cur, w = [], 0
        cur.append(r)
        w += wr
    chunks.append((cur, int(regoff64[cur[0]]) * F, w))

    # Split the first and last chunks at a region boundary: a small leading
    # chunk starts compute sooner (pipeline ramp), a small trailing chunk
    # shortens the drain.
    def _split(ch, at):
        regs, off, w = ch
        if len(regs) < 2:
            return [ch]
        a, b = regs[:at], regs[at:]
        wa = int(sum(C[r] for r in a)) * F
        return [(a, off, wa), (b, off + wa, w - wa)]

    def _explode(ch):
        regs, off, w = ch
        out = []
        o = off
        for r in regs:
            wr = int(C[r]) * F
            out.append(([r], o, wr))
            o += wr
        return out

    # fine-grained head and tail: the 4-stage pipeline fills and drains at
    # chunk granularity, so small chunks there cut the ramp/drain bubbles
    chunks = (_explode(chunks[0]) + _explode(chunks[1]) + chunks[2:-2]
              + _explode(chunks[-2]) + _explode(chunks[-1]))

    # Tier assignment by data volume (Bresenham over chunk widths),
    # interleaved so ACT and DVE chunks alternate through the timeline.
    # Chunk 0 stays tier B (HWDGE needs no Q7 boot).
    tiers = []
    acc_a, acc = 0.0, 0.0
    for i, (_, _, w) in enumerate(chunks):
        acc += w
        if i > 0 and acc_a + w <= ALPHA * acc + w / 2:
            tiers.append("A")
            acc_a += w
        else:
            tiers.append("B")

    # Per-edge slot: node rank -> (core, region, partition), edge -> slot k.
    rank_of = np.empty(N_NODES, np.int64)
    rank_of[order] = np.arange(N_NODES)
    erank = rank_of[dst]
    ecore = erank % NCORES
    eli = erank // NCORES
    er = eli // P
    ep = eli % P
    sidx = np.argsort(dst, kind="stable")
    starts = np.zeros(N_NODES + 1, np.int64)
    np.cumsum(deg, out=starts[1:])
    k_within = np.empty(N_EDGES, np.int64)
    k_within[sidx] = np.arange(N_EDGES) - starts[dst[sidx]]
    # index into the global [NCORES*128*F64] grid of 64-float blocks
    idx64 = ((ecore * P + ep) * F64 + regoff64[er] + k_within).astype(np.int64)

    # Per-node count-correction constants, per core: [128, 3*NREG].
    # The stats chain is uniform: mean = in0*kA; v1 = in1*kB;
    # var = v1 + mean^2*kC.  Per tier the (in0, in1) sources differ:
    #   B regions: in0 = sum, in1 = sumsq
    #     kA = 1/cnt; kB = 1/(cnt-1); kC = -cnt/(cnt-1)
    #   A regions: in0 = bn mean' (over Np padded elems), in1 = bn var'
    #     mean = mean'*Np/cnt; var = var'*Np/(cnt-1) + mean'^2*Np(1-Np/cnt)/(cnt-1)
    #     folded so the same chain applies with
    #     kA = Np/cnt; kB = Np/(cnt-1); kC = (Np/kA^2)(1-Np/cnt)/(cnt-1) ...
    #     expressed against mean (= mean'*kA): mean^2*kC with
    #     kC = (1 - Np/cnt) * (Np/(cnt-1)) / kA^2 = cnt(cnt/Np - 1)/(cnt-1)
    tier_of_region = {}
    for (regs, _, _), t in zip(chunks, tiers):
        for r in regs:
            tier_of_region[r] = t
    rr, pp = np.meshgrid(np.arange(NREG), np.arange(P), indexing="ij")
    li = rr * P + pp                                  # [NREG, P]
    dsort_pad = np.concatenate([dsort, np.zeros(NCORES * P * NREG, np.int64)])
    Np = (np.asarray(C, np.float64) * 64.0)[:, None]  # [NREG, 1]
    isA = np.array([tier_of_region[r] == "A" for r in range(NREG)])[:, None]
    consts = np.empty((NCORES, P, 3 * NREG), np.float32)
    for c in range(NCORES):
        cnt = (64.0 * dsort_pad[NCORES * li + c]).astype(np.float64)  # [NREG,P]
        m0 = np.maximum(cnt, 1.0)
        m1 = np.maximum(cnt - 1.0, 1.0)
        kA = np.where(isA, Np / m0, 1.0 / m0)
        kB = np.where(isA, Np / m1, 1.0 / m1)
        kC = np.where(isA, cnt * (cnt / Np - 1.0) / m1, -cnt / m1)
        consts[c, :, 0 * NREG:1 * NREG] = kA.T.astype(np.float32)
        consts[c, :, 1 * NREG:2 * NREG] = kB.T.astype(np.float32)
        consts[c, :, 2 * NREG:3 * NREG] = kC.T.astype(np.float32)

    return {
        "C": tuple(int(c) for c in C),
        "regoff64": regoff64,
        "F64": F64,
        "F_total": F_total,
        "chunks": chunks,
        "tiers": tuple(tiers),
        "idx64": idx64,
        "consts": consts,
    }


def _build(C, chunks, tiers, F_total):
    """Build the SPMD Bass program (one core's view)."""
    import concourse.bass as bass
    import concourse.mybir as mybir
    import concourse.tile as tile

    f32 = mybir.dt.float32
    bf16 = mybir.dt.bfloat16
    i8 = mybir.dt.int8
    Alu = mybir.AluOpType
    Act = mybir.ActivationFunctionType

    nc = bass.Bass()
    epad = nc.declare_dram_parameter("epad", [P, F_total], i8, isOutput=False)
    kon = nc.declare_dram_parameter("konst", [P, 3 * NREG], f32, isOutput=False)
    outp = nc.declare_dram_parameter("out", [P, F_total], i8, isOutput=True)

    regoff = np.zeros(NREG + 1, np.int64)
    np.cumsum(np.asarray(C, np.int64) * F, out=regoff[1:])
    wmax = int(max(chunks, key=lambda ch: ch[2])[2])

    with tile.TileContext(nc) as tc:
        with (
            tc.tile_pool(name="singles", bufs=1) as singles,
            tc.tile_pool(name="ioa", bufs=IO_BUFS_A) as ioa,
            tc.tile_pool(name="iob", bufs=IO_BUFS_B) as iob,
            tc.tile_pool(name="ota", bufs=OUT_BUFS_A) as ota,
            tc.tile_pool(name="otb", bufs=OUT_BUFS_B) as otb,
            tc.tile_pool(name="st", bufs=10) as st,
        ):
            ksb = singles.tile([P, 3 * NREG], f32)
            nc.sync.dma_start(out=ksb[:, :], in_=kon[:, :])
            # engine-private stride-0 dump tiles: the elementwise outputs
            # of the fused accumulate passes are never read, so broadcast
            # every write onto one column (frees SBUF + write bandwidth)
            sdump = singles.tile([P, 1], bf16)
            adump = singles.tile([P, 1], bf16)
            dbias = singles.tile([P, 1], f32)
            nc.vector.memset(dbias[:, :], VAR_DELTA / (OSCALE * OSCALE))

            def load(S):
                """Issue the chunk's input DMA (one iteration ahead)."""
                off, w = S["off"], S["w"]
                if S["tier"] == "B":
                    t = iob.tile([P, w], i8, tag="io8")
                    nc.sync.dma_start(out=t[:, :], in_=epad[:, off:off + w])
                else:
                    t = ioa.tile([P, w], bf16, tag="io16")
                    nc.gpsimd.dma_start(out=t[:, :], in_=epad[:, off:off + w])
                S["t"] = t

            def reduce_chunk(S):
                """Per-region reductions on the already-loading chunk."""
                regs, off, w, tier = S["regs"], S["off"], S["w"], S["tier"]
                t = S["t"]
                n = len(regs)
                if tier == "B":
                    sA = st.tile([P, n], f32, tag="sA")
                    ssA = st.tile([P, n], f32, tag="ssA")
                    for j, r in enumerate(regs):
                        o = int(regoff[r]) - off
                        wr = int(C[r]) * F
                        nc.scalar.activation(
                            out=adump[:, :].broadcast_to((P, wr)),
                            in_=t[:, o:o + wr], func=Act.Square,
                            accum_out=ssA[:, j:j + 1])
                    for j, r in enumerate(regs):
                        o = int(regoff[r]) - off
                        wr = int(C[r]) * F
                        h = wr // 2
                        nc.vector.scalar_tensor_tensor(
                            out=sdump[:, :].broadcast_to((P, h)),
                            in0=t[:, o:o + h], scalar=1.0,
                            in1=t[:, o + h:o + wr], op0=Alu.mult, op1=Alu.add,
                            accum_out=sA[:, j:j + 1])
                    S["in0"], S["in1"] = sA[:, :], ssA[:, :]
                else:
                    # bn_stats gives mean'/var' over each region (padded
                    # zeros included; host constants correct for that)
                    mv = st.tile([P, 2 * n], f32, tag="mv")
                    for j, r in enumerate(regs):
                        o = int(regoff[r]) - off
                        wr = int(C[r]) * F
                        ng = (wr + 511) // 512
                        bst = st.tile([P, ng, 6], f32, tag="bst")
                        g0 = 0
                        for g in range(ng):
                            gw = (wr // ng + 63) // 64 * 64 if g < ng - 1 \
                                else wr - g0
                            nc.vector.bn_stats(out=bst[:, g, :],
                                               in_=t[:, o + g0:o + g0 + gw])
                            g0 += gw
                        nc.vector.bn_aggr(out=mv[:, 2 * j:2 * j + 2],
                                          in_=bst[:, :, :])
                    S["in0"], S["in1"] = mv[:, 0:2 * n:2], mv[:, 1:2 * n:2]

            def stats1(S):
                """DVE: mean / var from the reduction outputs."""
                n = len(S["regs"])
                r0 = S["regs"][0]
                kAc = ksb[:, 0 * NREG + r0:0 * NREG + r0 + n]
                kBc = ksb[:, 1 * NREG + r0:1 * NREG + r0 + n]
                kCc = ksb[:, 2 * NREG + r0:2 * NREG + r0 + n]
                mean = st.tile([P, n], f32, tag="mean")
                nc.vector.tensor_mul(out=mean[:, :], in0=S["in0"], in1=kAc)
                v1 = st.tile([P, n], f32, tag="v1")
                nc.vector.tensor_mul(out=v1[:, :], in0=S["in1"], in1=kBc)
                msq = st.tile([P, n], f32, tag="msq")
                nc.vector.tensor_mul(out=msq[:, :], in0=mean[:, :],
                                     in1=mean[:, :])
                v2 = st.tile([P, n], f32, tag="v2")
                nc.vector.tensor_mul(out=v2[:, :], in0=msq[:, :], in1=kCc)
                var = st.tile([P, n], f32, tag="var")
                nc.vector.tensor_add(out=var[:, :], in0=v1[:, :], in1=v2[:, :])
                S["mean"], S["var"] = mean, var

            def act_sqrt(S):
                """ACT: std32 = sqrt((var+delta))/OSCALE (scale folded in)."""
                n = len(S["regs"])
                std = st.tile([P, n], f32, tag="std")
                nc.scalar.activation(out=std[:, :], in_=S["var"][:, :],
                                     func=Act.Sqrt, scale=1.0 / (OSCALE * OSCALE),
                                     bias=dbias[:, 0:1])
                S["std"] = std

            def stats2(S):
                """DVE: svec = 1/std32 = OSCALE/std; bvec = -mean*svec."""
                n = len(S["regs"])
                svec = st.tile([P, n], f32, tag="svec")
                nc.vector.reciprocal(out=svec[:, :], in_=S["std"][:, :])
                bvec = st.tile([P, n], f32, tag="bvec")
                nc.vector.scalar_tensor_tensor(
                    out=bvec[:, :], in0=S["mean"][:, :], scalar=-1.0,
                    in1=svec[:, :], op0=Alu.mult, op1=Alu.mult)
                S["svec"], S["bvec"] = svec, bvec

            def norm_store(S):
                """Normalize+quantize and stream the chunk out."""
                t, regs, off, w = S["t"], S["regs"], S["off"], S["w"]
                svec, bvec, mean = S["svec"], S["bvec"], S["mean"]
                if S["tier"] == "B":
                    o8 = otb.tile([P, w], i8, tag="o8")
                    for j, r in enumerate(regs):
                        o = int(regoff[r]) - off
                        wr = int(C[r]) * F
                        nc.scalar.activation(
                            out=o8[:, o:o + wr], in_=t[:, o:o + wr],
                            func=Act.Identity, bias=bvec[:, j:j + 1],
                            scale=svec[:, j:j + 1])
                    nc.sync.dma_start(out=outp[:, off:off + w], in_=o8[:, :])
                else:
                    o16 = ota.tile([P, w], bf16, tag="o16")
                    for j, r in enumerate(regs):
                        o = int(regoff[r]) - off
                        wr = int(C[r]) * F
                        # out = (q - mean) * svec == q*svec + bvec
                        nc.vector.tensor_scalar(
                            out=o16[:, o:o + wr], in0=t[:, o:o + wr],
                            scalar1=mean[:, j:j + 1], scalar2=svec[:, j:j + 1],
                            op0=Alu.subtract, op1=Alu.mult)
                    nc.gpsimd.dma_start(out=outp[:, off:off + w], in_=o16[:, :])

            # 4-stage software pipeline: every cross-engine dependency has
            # a full chunk-iteration of slack, so neither in-order engine
            # FIFO head-of-line blocks on the other engine.
            states = [{"regs": regs, "off": off, "w": w, "tier": tiers[ci]}
                      for ci, (regs, off, w) in enumerate(chunks)]
            nch = len(chunks)
            load(states[0])
            for ci in range(nch):
                if ci >= 3:
                    norm_store(states[ci - 3])
                if ci >= 1:
                    stats1(states[ci - 1])
                    act_sqrt(states[ci - 1])
                if ci >= 2:
                    stats2(states[ci - 2])
                if ci + 1 < nch:
                    load(states[ci + 1])
                reduce_chunk(states[ci])
            stats1(states[nch - 1])
            act_sqrt(states[nch - 1])
            stats2(states[nch - 2])
            stats2(states[nch - 1])
            for k in range(max(0, nch - 3), nch):
                norm_store(states[k])

    _split_multiwaits(nc)
    return nc


def _pack(plan, e):
    """f32 edges -> per-core padded int8 matrices [NCORES, P, F_total]."""
    F64, F_total, idx64 = plan["F64"], plan["F_total"], plan["idx64"]
    e = np.asarray(e, np.float32)
    q = np.clip(np.rint(e * QSCALE), -127, 127).astype(np.int8)
    epad = np.zeros((NCORES * P * F64, F), np.int8)
    epad[idx64] = q
    return epad.reshape(NCORES, P, F_total)


def _make_in_maps(plan, e):
    """Build per-core input dicts (epad, konst)."""
    epad = _pack(plan, e)
    return [{"epad": epad[c], "konst": plan["consts"][c]}
            for c in range(NCORES)]


def _unpack(plan, res, gamma, beta):
    """Gather per-core int8 outputs back to [N_EDGES, F] f32; apply the
    dequant scale and gamma/beta as an exact host epilogue."""
    F64 = plan["F64"]
    out_pad = np.stack([res.results[c]["out"] for c in range(NCORES)])
    out = out_pad.reshape(NCORES * P * F64, F)[plan["idx64"]]
    g = (gamma / OSCALE).astype(np.float32)
    return out.astype(np.float32) * g + beta.astype(np.float32)


def kernel(e, gamma, beta, dst):
    _install_ntff_hook()
    from concourse.bass_utils import run_bass_kernel_spmd

    gamma = np.asarray(gamma, dtype=np.float32)
    beta = np.asarray(beta, dtype=np.float32)
    dst_i = np.asarray(dst)

    key = hash(dst_i.tobytes())
    plan = _PLAN_CACHE.get(key)
    if plan is None:
        plan = _plan(dst_i)
        _PLAN_CACHE[key] = plan

    bkey = (plan["C"], plan["tiers"])
    nc = _BUILD_CACHE.get(bkey)
    if nc is None:
        nc = _build(plan["C"], plan["chunks"], plan["tiers"], plan["F_total"])
        _BUILD_CACHE[bkey] = nc

    in_maps = _make_in_maps(plan, e)
    res = run_bass_kernel_spmd(nc, in_maps, core_ids=list(range(NCORES)))
    return _unpack(plan, res, gamma, beta)


# revision 14
# speedup vs baseline: 1.0204x; 1.0204x over previous
"""Trainium2 kernel for AdjaEdgeNorm: per-destination-node edge-mailbox
normalization (mean/std over each dst node's incoming edge features).

Strategy (follows the sharding hint):
  - Host: partition the graph by destination node. Nodes are sorted by
    degree (desc) and dealt round-robin to the 8 cores, so every core has
    the same degree profile. Each core's 6250 nodes are grouped into 49
    regions of 128 nodes; a region's nodes are padded to the region max
    degree C_r rounded up to a multiple of 2. Each core's data is one
    [128, sum_r C_r*64] int8 matrix (q = round(32*x); the scale cancels
    in the normalization): partition p of region r holds node (r,p)'s
    padded edge mailbox, flattened.
  - Device (SPMD, one NEFF on 8 cores, zero cross-core communication),
    two chunk tiers balancing ACT vs DVE vs DMA:
      B-chunks (pure-ACT): raw int8 HWDGE load; ACT Square+accum (sum of
        squares, int8 read directly) and ACT Identity with per-node AP
        scale/bias -> int8 normalized output (fused quantize, RNE); DVE
        does only the row-sum via one fused scalar_tensor_tensor
        (left half + right half, accumulated).
      A-chunks (pure-DVE): SWDGE int8->bf16 cast load; DVE STT row-sum,
        STT self-mult sum-of-squares, tensor_scalar (sub,mult) normalize
        out-of-place -> bf16 (4x mode); SWDGE cast bf16->int8 store.
    Per-chunk stats on DVE with host-precomputed count-correction
    constants; sqrt(var + delta) on ACT replaces max(var,0)+eps (the
    reference's +eps only matters for degenerate nodes whose output is 0
    either way).
  - Host: gather int8 output back to edge order; apply gamma/beta (and
    the 1/32 dequant) as an exact elementwise epilogue.
"""

import sys
import types

import numpy as np

N_NODES = 50000
N_EDGES = 1600000
F = 64
EPS = 1e-5
QSCALE = 32.0
OSCALE = 32.0
NCORES = 8
P = 128
NODES_PER_CORE = N_NODES // NCORES          # 6250
NREG = (NODES_PER_CORE + P - 1) // P        # 49
CHUNK_W_MAX = 8192                          # elems/partition per chunk DMA
IO_BUFS_A = 5
IO_BUFS_B = 5
OUT_BUFS_A = 3
OUT_BUFS_B = 3
ALPHA = 0.46                                # fraction of data on the A (DVE) tier
VAR_DELTA = 1.0                             # sqrt(var + delta) guard, q^2 units

_PLAN_CACHE = {}
_BUILD_CACHE = {}


def _bf16():
    import ml_dtypes
    return np.dtype(ml_dtypes.bfloat16)


def _install_ntff_hook():
    """The agent container's antenv stub lacks axon_hooks; recreate it so
    run_bass_kernel_spmd(trace=True) can capture NTFF profiles. Harmless
    if unavailable."""
    if "antenv.axon_hooks" in sys.modules:
        return
    try:
        from trn_agent_boot.trn_boot import _ntff_profile_via_ctypes
        hook = _ntff_profile_via_ctypes("/opt/axon/libaxon_pjrt.so")
    except Exception:
        hook = None
    mod = types.ModuleType("antenv.axon_hooks")
    mod.get_axon_ntff_profile_hook = lambda: hook
    mod.set_axon_ntff_profile_hook = lambda h: None
    sys.modules["antenv.axon_hooks"] = mod


def _split_multiwaits(nc):
    """walrus in this container supports a single sync-wait per instruction;
    Tile's tail drain can carry one wait per DMA lane. Hoist extras onto
    standalone NoOps on the same engine, just before the instruction."""
    import concourse.mybir as mybir

    k = 0
    for f in nc.m.functions:
        for bb in f.blocks:
            new = []
            for inst in bb.instructions:
                si = inst.sync_info
                if si is not None and si.on_wait is not None and len(si.on_wait) > 1:
                    for w in si.on_wait[:-1]:
                        nop = mybir.InstNoOp(name=f"I-mwsplit-{k}", ins=[], outs=[])
                        k += 1
                        nop.engine = inst.engine
                        nop.sync_info = mybir.SyncInfo(on_wait=[w], on_update=[])
                        new.append(nop)
                    si.on_wait = si.on_wait[-1:]
                new.append(inst)
            bb.instructions[:] = new


def _plan(dst):
    """All index preprocessing derived from dst alone."""
    dst = np.asarray(dst, dtype=np.int64)
    deg = np.bincount(dst, minlength=N_NODES)
    order = np.argsort(-deg, kind="stable")          # node ids, degree desc
    dsort = deg[order]

    # Region widths: region r spans global degree-ranks [1024r, 1024r+1024).
    # Rounded up to a multiple of 2 so the fused half+half row-sum stays
    # region-local.
    C = np.empty(NREG, np.int64)
    for r in range(NREG):
        c = max(int(dsort[min(1024 * r, N_NODES - 1)]), 1)
        C[r] = (c + 1) // 2 * 2
    regoff64 = np.zeros(NREG + 1, np.int64)
    np.cumsum(C, out=regoff64[1:])                   # region start, 64-blocks
    F64 = int(regoff64[-1])
    F_total = F64 * F

    # Chunks: consecutive regions grouped so each chunk DMA is big.
    chunks = []  # (list_of_regions, off_floats, width_floats)
    cur, w = [], 0
    for r in range(NREG):
        wr = int(C[r]) * F
        if w + wr > CHUNK_W_MAX and cur:
            chunks.append((cur, int(regoff64[cur[0]]) * F, w))
            cur, w = [], 0
        cur.append(r)
        w += wr
    chunks.append((cur, int(regoff64[cur[0]]) * F, w))

    # Split the first and last chunks at a region boundary: a small leading
    # chunk starts compute sooner (pipeline ramp), a small trailing chunk
    # shortens the drain.
    def _split(ch, at):
        regs, off, w = ch
        if len(regs) < 2:
            return [ch]
        a, b = regs[:at], regs[at:]
        wa = int(sum(C[r] for r in a)) * F
        return [(a, off, wa), (b, off + wa, w - wa)]

    def _explode(ch):
        regs, off, w = ch
        out = []
        o = off
        for r in regs:
            wr = int(C[r]) * F
            out.append(([r], o, wr))
            o += wr
        return out

    # fine-grained head and tail: the 4-stage pipeline fills and drains at
    # chunk granularity, so small chunks there cut the ramp/drain bubbles
    chunks = (_explode(chunks[0]) + _explode(chunks[1]) + chunks[2:-2]
              + _explode(chunks[-2]) + _explode(chunks[-1]))

    # Tier assignment by data volume (Bresenham over chunk widths),
    # interleaved so ACT and DVE chunks alternate through the timeline.
    # Chunk 0 stays tier B (HWDGE needs no Q7 boot).
    tiers = []
    acc_a, acc = 0.0, 0.0
    for i, (_, _, w) in enumerate(chunks):
        acc += w
        if i > 0 and acc_a + w <= ALPHA * acc + w / 2:
            tiers.append("A")
            acc_a += w
        else:
            tiers.append("B")

    # Per-edge slot: node rank -> (core, region, partition), edge -> slot k.
    rank_of = np.empty(N_NODES, np.int64)
    rank_of[order] = np.arange(N_NODES)
    erank = rank_of[dst]
    ecore = erank % NCORES
    eli = erank // NCORES
    er = eli // P
    ep = eli % P
    sidx = np.argsort(dst, kind="stable")
    starts = np.zeros(N_NODES + 1, np.int64)
    np.cumsum(deg, out=starts[1:])
    k_within = np.empty(N_EDGES, np.int64)
    k_within[sidx] = np.arange(N_EDGES) - starts[dst[sidx]]
    # index into the global [NCORES*128*F64] grid of 64-float blocks
    idx64 = ((ecore * P + ep) * F64 + regoff64[er] + k_within).astype(np.int64)

    # Per-node count-correction constants, per core: [128, 3*NREG].
    # The stats chain is uniform: mean = in0*kA; v1 = in1*kB;
    # var = v1 + mean^2*kC.  Per tier the (in0, in1) sources differ:
    #   B regions: in0 = sum, in1 = sumsq
    #     kA = 1/cnt; kB = 1/(cnt-1); kC = -cnt/(cnt-1)
    #   A regions: in0 = bn mean' (over Np padded elems), in1 = bn var'
    #     mean = mean'*Np/cnt; var = var'*Np/(cnt-1) + mean'^2*Np(1-Np/cnt)/(cnt-1)
    #     folded so the same chain applies with
    #     kA = Np/cnt; kB = Np/(cnt-1); kC = (Np/kA^2)(1-Np/cnt)/(cnt-1) ...
    #     expressed against mean (= mean'*kA): mean^2*kC with
    #     kC = (1 - Np/cnt) * (Np/(cnt-1)) / kA^2 = cnt(cnt/Np - 1)/(cnt-1)
    tier_of_region = {}
    for (regs, _, _), t in zip(chunks, tiers):
        for r in regs:
            tier_of_region[r] = t
    rr, pp = np.meshgrid(np.arange(NREG), np.arange(P), indexing="ij")
    li = rr * P + pp                                  # [NREG, P]
    dsort_pad = np.concatenate([dsort, np.zeros(NCORES * P * NREG, np.int64)])
    Np = (np.asarray(C, np.float64) * 64.0)[:, None]  # [NREG, 1]
    isA = np.array([tier_of_region[r] == "A" for r in range(NREG)])[:, None]
    consts = np.empty((NCORES, P, 3 * NREG), np.float32)
    for c in range(NCORES):
        cnt = (64.0 * dsort_pad[NCORES * li + c]).astype(np.float64)  # [NREG,P]
        m0 = np.maximum(cnt, 1.0)
        m1 = np.maximum(cnt - 1.0, 1.0)
        kA = np.where(isA, Np / m0, 1.0 / m0)
        kB = np.where(isA, Np / m1, 1.0 / m1)
        kC = np.where(isA, cnt * (cnt / Np - 1.0) / m1, -cnt / m1)
        consts[c, :, 0 * NREG:1 * NREG] = kA.T.astype(np.float32)
        consts[c, :, 1 * NREG:2 * NREG] = kB.T.astype(np.float32)
        consts[c, :, 2 * NREG:3 * NREG] = kC.T.astype(np.float32)

    return {
        "C": tuple(int(c) for c in C),
        "regoff64": regoff64,
        "F64": F64,
        "F_total": F_total,
        "chunks": chunks,
        "tiers": tuple(tiers),
        "idx64": idx64,
        "consts": consts,
    }


def _build(C, chunks, tiers, F_total):
    """Build the SPMD Bass program (one core's view)."""
    import concourse.bass as bass
    import concourse.mybir as mybir
    import concourse.tile as tile

    f32 = mybir.dt.float32
    bf16 = mybir.dt.bfloat16
    i8 = mybir.dt.int8
    Alu = mybir.AluOpType
    Act = mybir.ActivationFunctionType

    nc = bass.Bass()
    epad = nc.declare_dram_parameter("epad", [P, F_total], i8, isOutput=False)
    kon = nc.declare_dram_parameter("konst", [P, 3 * NREG], f32, isOutput=False)
    outp = nc.declare_dram_parameter("out", [P, F_total], i8, isOutput=True)

    regoff = np.zeros(NREG + 1, np.int64)
    np.cumsum(np.asarray(C, np.int64) * F, out=regoff[1:])
    wmax = int(max(chunks, key=lambda ch: ch[2])[2])

    with tile.TileContext(nc) as tc:
        with (
            tc.tile_pool(name="singles", bufs=1) as singles,
            tc.tile_pool(name="ioa", bufs=IO_BUFS_A) as ioa,
            tc.tile_pool(name="iob", bufs=IO_BUFS_B) as iob,
            tc.tile_pool(name="ota", bufs=OUT_BUFS_A) as ota,
            tc.tile_pool(name="otb", bufs=OUT_BUFS_B) as otb,
            tc.tile_pool(name="st", bufs=10) as st,
        ):
            ksb = singles.tile([P, 3 * NREG], f32)
            nc.sync.dma_start(out=ksb[:, :], in_=kon[:, :])
            # engine-private stride-0 dump tiles: the elementwise outputs
            # of the fused accumulate passes are never read, so broadcast
            # every write onto one column (frees SBUF + write bandwidth)
            sdump = singles.tile([P, 1], bf16)
            adump = singles.tile([P, 1], bf16)
            dbias = singles.tile([P, 1], f32)
            nc.vector.memset(dbias[:, :], VAR_DELTA / (OSCALE * OSCALE))

            def load(S):
                """Issue the chunk's input DMA (one iteration ahead)."""
                off, w = S["off"], S["w"]
                if S["tier"] == "B":
                    t = iob.tile([P, w], i8, tag="io8")
                    nc.sync.dma_start(out=t[:, :], in_=epad[:, off:off + w])
                else:
                    t = ioa.tile([P, w], bf16, tag="io16")
                    nc.gpsimd.dma_start(out=t[:, :], in_=epad[:, off:off + w])
                S["t"] = t

            def reduce_chunk(S):
                """Per-region reductions on the already-loading chunk."""
                regs, off, w, tier = S["regs"], S["off"], S["w"], S["tier"]
                t = S["t"]
                n = len(regs)
                if tier == "B":
                    sA = st.tile([P, n], f32, tag="sA")
                    ssA = st.tile([P, n], f32, tag="ssA")
                    for j, r in enumerate(regs):
                        o = int(regoff[r]) - off
                        wr = int(C[r]) * F
                        nc.scalar.activation(
                            out=adump[:, :].broadcast_to((P, wr)),
                            in_=t[:, o:o + wr], func=Act.Square,
                            accum_out=ssA[:, j:j + 1])
                    for j, r in enumerate(regs):
                        o = int(regoff[r]) - off
                        wr = int(C[r]) * F
                        h = wr // 2
                        nc.vector.scalar_tensor_tensor(
                            out=sdump[:, :].broadcast_to((P, h)),
                            in0=t[:, o:o + h], scalar=1.0,
                            in1=t[:, o + h:o + wr], op0=Alu.mult, op1=Alu.add,
                            accum_out=sA[:, j:j + 1])
                    S["in0"], S["in1"] = sA[:, :], ssA[:, :]
                else:
                    # bn_stats gives mean'/var' over each region (padded
                    # zeros included; host constants correct for that)
                    mv = st.tile([P, 2 * n], f32, tag="mv")
                    for j, r in enumerate(regs):
                        o = int(regoff[r]) - off
                        wr = int(C[r]) * F
                        ng = (wr + 511) // 512
                        bst = st.tile([P, ng, 6], f32, tag="bst")
                        g0 = 0
                        for g in range(ng):
                            gw = (wr // ng + 63) // 64 * 64 if g < ng - 1 \
                                else wr - g0
                            nc.vector.bn_stats(out=bst[:, g, :],
                                               in_=t[:, o + g0:o + g0 + gw])
                            g0 += gw
                        nc.vector.bn_aggr(out=mv[:, 2 * j:2 * j + 2],
                                          in_=bst[:, :, :])
                    S["in0"], S["in1"] = mv[:, 0:2 * n:2], mv[:, 1:2 * n:2]

            def stats1(S):
                """DVE: mean / var from the reduction outputs."""
                n = len(S["regs"])
                r0 = S["regs"][0]
                kAc = ksb[:, 0 * NREG + r0:0 * NREG + r0 + n]
                kBc = ksb[:, 1 * NREG + r0:1 * NREG + r0 + n]
                kCc = ksb[:, 2 * NREG + r0:2 * NREG + r0 + n]
                mean = st.tile([P, n], f32, tag="mean")
                nc.vector.tensor_mul(out=mean[:, :], in0=S["in0"], in1=kAc)
                v1 = st.tile([P, n], f32, tag="v1")
                nc.vector.tensor_mul(out=v1[:, :], in0=S["in1"], in1=kBc)
                msq = st.tile([P, n], f32, tag="msq")
                nc.vector.tensor_mul(out=msq[:, :], in0=mean[:, :],
                                     in1=mean[:, :])
                v2 = st.tile([P, n], f32, tag="v2")
                nc.vector.tensor_mul(out=v2[:, :], in0=msq[:, :], in1=kCc)
                var = st.tile([P, n], f32, tag="var")
                nc.vector.tensor_add(out=var[:, :], in0=v1[:, :], in1=v2[:, :])
                S["mean"], S["var"] = mean, var

            def act_sqrt(S):
                """ACT: std32 = sqrt((var+delta))/OSCALE (scale folded in)."""
                n = len(S["regs"])
                std = st.tile([P, n], f32, tag="std")
                nc.scalar.activation(out=std[:, :], in_=S["var"][:, :],
                                     func=Act.Sqrt, scale=1.0 / (OSCALE * OSCALE),
                                     bias=dbias[:, 0:1])
                S["std"] = std

            def stats2(S):
                """DVE: svec = 1/std32 = OSCALE/std; bvec = -mean*svec."""
                n = len(S["regs"])
                svec = st.tile([P, n], f32, tag="svec")
                nc.vector.reciprocal(out=svec[:, :], in_=S["std"][:, :])
                bvec = st.tile([P, n], f32, tag="bvec")
                nc.vector.scalar_tensor_tensor(
                    out=bvec[:, :], in0=S["mean"][:, :], scalar=-1.0,
                    in1=svec[:, :], op0=Alu.mult, op1=Alu.mult)
                S["svec"], S["bvec"] = svec, bvec

            def norm_store(S):
                """Normalize+quantize and stream the chunk out."""
                t, regs, off, w = S["t"], S["regs"], S["off"], S["w"]
                svec, bvec, mean = S["svec"], S["bvec"], S["mean"]
                if S["tier"] == "B":
                    o8 = otb.tile([P, w], i8, tag="o8")
                    for j, r in enumerate(regs):
                        o = int(regoff[r]) - off
                        wr = int(C[r]) * F
                        nc.scalar.activation(
                            out=o8[:, o:o + wr], in_=t[:, o:o + wr],
                            func=Act.Identity, bias=bvec[:, j:j + 1],
                            scale=svec[:, j:j + 1])
                    nc.sync.dma_start(out=outp[:, off:off + w], in_=o8[:, :])
                else:
                    o16 = ota.tile([P, w], bf16, tag="o16")
                    for j, r in enumerate(regs):
                        o = int(regoff[r]) - off
                        wr = int(C[r]) * F
                        # out = (q - mean) * svec == q*svec + bvec
                        nc.vector.tensor_scalar(
                            out=o16[:, o:o + wr], in0=t[:, o:o + wr],
                            scalar1=mean[:, j:j + 1], scalar2=svec[:, j:j + 1],
                            op0=Alu.subtract, op1=Alu.mult)
                    nc.gpsimd.dma_start(out=outp[:, off:off + w], in_=o16[:, :])

            # 4-stage software pipeline: every cross-engine dependency has
            # a full chunk-iteration of slack, so neither in-order engine
            # FIFO head-of-line blocks on the other engine.
            states = [{"regs": regs, "off": off, "w": w, "tier": tiers[ci]}
                      for ci, (regs, off, w) in enumerate(chunks)]
            nch = len(chunks)
            load(states[0])
            for ci in range(nch):
                if ci >= 1:
                    stats1(states[ci - 1])
                    act_sqrt(states[ci - 1])
                if ci >= 2:
                    stats2(states[ci - 2])
                if ci >= 3:
                    norm_store(states[ci - 3])
                if ci + 1 < nch:
                    load(states[ci + 1])
                reduce_chunk(states[ci])
            stats1(states[nch - 1])
            act_sqrt(states[nch - 1])
            stats2(states[nch - 2])
            stats2(states[nch - 1])
            for k in range(max(0, nch - 3), nch):
                norm_store(states[k])

    _split_multiwaits(nc)
    return nc


def _pack(plan, e):
    """f32 edges -> per-core padded int8 matrices [NCORES, P, F_total]."""
    F64, F_total, idx64 = plan["F64"], plan["F_total"], plan["idx64"]
    e = np.asarray(e, np.float32)
    q = np.clip(np.rint(e * QSCALE), -127, 127).astype(np.int8)
    epad = np.zeros((NCORES * P * F64, F), np.int8)
    epad[idx64] = q
    return epad.reshape(NCORES, P, F_total)


def _make_in_maps(plan, e):
    """Build per-core input dicts (epad, konst)."""
    epad = _pack(plan, e)
    return [{"epad": epad[c], "konst": plan["consts"][c]}
            for c in range(NCORES)]


def _unpack(plan, res, gamma, beta):
    """Gather per-core int8 outputs back to [N_EDGES, F] f32; apply the
    dequant scale and gamma/beta as an exact host epilogue."""
    F64 = plan["F64"]
    out_pad = np.stack([res.results[c]["out"] for c in range(NCORES)])
    out = out_pad.reshape(NCORES * P * F64, F)[plan["idx64"]]
    g = (gamma / OSCALE).astype(np.float32)
    return out.astype(np.float32) * g + beta.astype(np.float32)


def kernel(e, gamma, beta, dst):
    _install_ntff_hook()
    from concourse.bass_utils import run_bass_kernel_spmd

    gamma = np.asarray(gamma, dtype=np.float32)
    beta = np.asarray(beta, dtype=np.float32)
    dst_i = np.asarray(dst)

    key = hash(dst_i.tobytes())
    plan = _PLAN_CACHE.get(key)
    if plan is None:
        plan = _plan(dst_i)
        _PLAN_CACHE[key] = plan

    bkey = (plan["C"], plan["tiers"])
    nc = _BUILD_CACHE.get(bkey)
    if nc is None:
        nc = _build(plan["C"], plan["chunks"], plan["tiers"], plan["F_total"])
        _BUILD_CACHE[bkey] = nc

    in_maps = _make_in_maps(plan, e)
    res = run_bass_kernel_spmd(nc, in_maps, core_ids=list(range(NCORES)))
    return _unpack(plan, res, gamma, beta)
